# revision 17
# baseline (speedup 1.0000x reference)
"""A3C ChebConv (K=3) GNN model as a distributed Bass kernel on 8 TRN2 cores.

Math restructuring: the reference computes
    Tx0 = x; Tx1 = L@x; Tx2 = 2*L@Tx1 - x
    out = Tx0@W0 + Tx1@W1 + Tx2@W2 + b
Since L acts on the node dim and W on the feature dim, they commute:
    out = Y0 - Y2 + b + L@(Y1 + 2*L@Y2),   Y_k = x@W_k
So the only big compute is x@W (feature contraction, F=65536), which is
sharded over F across 8 cores; the [100, 360] partial products are
all-reduced, and the tiny Laplacian/tanh/FC epilogue runs on every core.

Per-core device graph:
  - one fused matmul  xT_shard[8192,100].T @ W_shard[8192,360] -> PSUM[100,360]
    (64 K-tiles of 128, streamed from one interleaved DRAM buffer)
  - AllReduce[100,360] over 8 cores
  - epilogue: U = L@Y2; Vin = Y1 + 2U; Z = bias + L@Vin + Y0 - Y2;
    emb = tanh(Z); FC heads via 60 accumulating [K=100,M=2]x[K=100,N=101]
    matmuls + one extras/bias matmul -> out[2,101]
"""

import numpy as np

import concourse.bass as bass
import concourse.bacc as bacc
import concourse.mybir as mybir
from concourse import tile
from concourse.tile_rust import add_dep_helper

N_CORES = 8
N = 100          # nodes
F = 65536        # input features
FS = F // N_CORES  # features per core
C = 60           # conv channels per head
CB = 2 * C       # both heads interleaved [actor | critic] per Cheb order
NW = 6 * C       # 360 = fused W columns (3 cheb orders x 2 heads)
BB = NW + N      # 460 = bigbuf row: [W row | xT row]
KT = 128         # contraction tile
NKT = FS // KT   # 64 K tiles per core
CHUNKS = 8       # DMA chunks (NKT/CHUNKS tiles each)
TPC = NKT // CHUNKS
ACT = 100        # action dim
FCN = ACT + 1    # fused FC output cols: [logits | value]
SM_COLS = 224    # smalls tensor cols

F32 = mybir.dt.float32
BF16 = mybir.dt.bfloat16
# Ship the big streamed tensors (x/W interleave + FC weights) in bf16:
# halves HBM traffic and runs the TensorEngine at 1 cycle/row (vs 4 for f32).
MM_BF16 = True
MMDT = BF16 if MM_BF16 else F32
# Wake the collective firmware early with a tiny dummy AllReduce that runs
# under the streaming phase, so the real AllReduce doesn't pay the ~11us
# ncfw wake latency.
PREWARM_CC = True
# K-tiles per DMA chunk, front-loaded small so the TensorEngine starts early.
CHUNK_SIZES = [2, 2, 4, 8, 8, 8, 8, 8, 8, 8]
assert sum(CHUNK_SIZES) == NKT


def build_nc(debug: bool = False, reps: int = 1):
    nc = bacc.Bacc(
        "TRN2", target_bir_lowering=False, debug=debug, num_devices=N_CORES
    )
    bigbuf = nc.dram_tensor("bigbuf", [128, NKT * BB], MMDT, kind="ExternalInput")
    fcw = nc.dram_tensor("fcw", [N, C * FCN], MMDT, kind="ExternalInput")
    lt = nc.dram_tensor("lt", [N, N], F32, kind="ExternalInput")
    smalls = nc.dram_tensor("smalls", [4, SM_COLS], F32, kind="ExternalInput")
    out_ext = nc.dram_tensor("out", [2, FCN], F32, kind="ExternalOutput")

    with tile.TileContext(nc) as tc:
        with (
            tc.tile_pool(name="big", bufs=CHUNKS) as bigpool,
            tc.tile_pool(name="wk", bufs=1) as wk,
            tc.tile_pool(name="ps", bufs=1, space="PSUM") as ps,
            tc.tile_pool(name="dram", bufs=1, space="DRAM") as dram,
        ):
            for _rep in range(reps):
                _build_body(nc, bigpool, wk, ps, dram, bigbuf, fcw, lt, smalls, out_ext)

    nc.compile()
    return nc


def _build_body(nc, bigpool, wk, ps, dram, bigbuf, fcw, lt, smalls, out_ext):
    if True:
        if True:
            # Small persistent tensors. Scheduled BEFORE the streaming chunks
            # (via explicit order-deps below): if these land mid-AllReduce,
            # they contend with the collective's SDMA traffic and stall the
            # FC phase behind a late fcw arrival.
            fcw_s = wk.tile([N, C * FCN], MMDT, tag="fcw")
            i_fcw = nc.sync.dma_start(fcw_s[:], fcw[:, :])
            lt_s = wk.tile([N, N], F32, tag="lt")
            i_lt = nc.sync.dma_start(lt_s[:], lt[:, :])
            sm_s = wk.tile([4, SM_COLS], F32, tag="smalls")
            i_sm = nc.sync.dma_start(sm_s[:], smalls[:, :])
            ones_s = wk.tile([1, N], F32, tag="ones")
            nc.any.memset(ones_s[:], 1.0)

            if PREWARM_CC:
                warm_in = dram.tile([1, 8], F32, tag="warmin")
                warm_out = dram.tile([1, 8], F32, tag="warmout")
                warm_sb = wk.tile([1, 8], F32, tag="warmsb")
                nc.any.memset(warm_sb[:], 0.0)
                nc.gpsimd.dma_start(warm_in[:], warm_sb[:])
                nc.gpsimd.collective_compute(
                    "AllReduce",
                    mybir.AluOpType.add,
                    replica_groups=[list(range(N_CORES))],
                    ins=[warm_in.opt()],
                    outs=[warm_out.opt()],
                )

            # Big fused matmul: accumulate all 64 K-tiles into one PSUM bank
            psum_y = ps.tile([N, NW], F32, tag="y")
            mm = 0
            lo = 0
            chunk_dmas = []
            for ch, tpc in enumerate(CHUNK_SIZES):
                bt = bigpool.tile([128, tpc * BB], MMDT, tag=f"chunk{min(ch,3)}")
                i_ch = nc.sync.dma_start(bt[:], bigbuf[:, lo : lo + tpc * BB])
                chunk_dmas.append(i_ch)
                lo += tpc * BB
                for t in range(tpc):
                    base = t * BB
                    lhsT = bt[:, base + NW : base + BB]  # [128, 100] xT tile
                    rhs = bt[:, base : base + NW]        # [128, 360] W tile
                    nc.tensor.matmul(
                        psum_y[:], lhsT, rhs,
                        start=(mm == 0), stop=(mm == NKT - 1),
                    )
                    mm += 1
            # Order-only edges: ramp chunks 0-2 first, then the persistent
            # tensors (so they land well before the AllReduce window), then
            # the remaining chunks.
            for small_dma in (i_fcw, i_lt, i_sm):
                add_dep_helper(
                    small_dma.ins, chunk_dmas[2].ins, False,
                    "persistent loads after the DMA ramp",
                )
                add_dep_helper(
                    chunk_dmas[3].ins, small_dma.ins, False,
                    "persistent loads before the stream tail",
                )

            # Evict partials and AllReduce across the 8 cores
            y_part = wk.tile([N, NW], F32, tag="ypart")
            nc.vector.tensor_copy(y_part[:], psum_y[:])
            ar_in = dram.tile([N, NW], F32, tag="arin")
            ar_out = dram.tile([N, NW], F32, tag="arout")
            nc.sync.dma_start(ar_in[:], y_part[:])
            nc.gpsimd.collective_compute(
                "AllReduce",
                mybir.AluOpType.add,
                replica_groups=[list(range(N_CORES))],
                ins=[ar_in.opt()],
                outs=[ar_out.opt()],
            )
            y_s = wk.tile([N, NW], F32, tag="ysb")
            nc.sync.dma_start(y_s[:], ar_out[:])

            # Epilogue: U = L@Y2
            psum_u = ps.tile([N, CB], F32, tag="u")
            nc.tensor.matmul(
                psum_u[:], lt_s[:], y_s[:, 2 * CB : 3 * CB], start=True, stop=True
            )
            # Vin = 2*U + Y1
            vin_s = wk.tile([N, CB], F32, tag="vin")
            nc.vector.scalar_tensor_tensor(
                vin_s[:], psum_u[:], 2.0, y_s[:, CB : 2 * CB],
                op0=mybir.AluOpType.mult, op1=mybir.AluOpType.add,
            )
            # Z = bias (ones[1,100].T @ biasrow[1,120]) + L@Vin
            psum_z = ps.tile([N, CB], F32, tag="z")
            nc.tensor.matmul(
                psum_z[:], ones_s[:], sm_s[0:1, 103 : 103 + CB],
                start=True, stop=False, skip_group_check=True,
            )
            nc.tensor.matmul(
                psum_z[:], lt_s[:], vin_s[:],
                start=False, stop=True, skip_group_check=True,
            )
            # emb = tanh(Z + Y0 - Y2)
            d_s = wk.tile([N, CB], F32, tag="d")
            nc.vector.tensor_sub(d_s[:], y_s[:, 0:CB], y_s[:, 2 * CB : 3 * CB])
            z_s = wk.tile([N, CB], F32, tag="zs")
            nc.vector.tensor_add(z_s[:], d_s[:], psum_z[:])
            emb_s = wk.tile([N, CB], MMDT, tag="emb")
            nc.scalar.activation(
                emb_s[:], z_s[:], mybir.ActivationFunctionType.Tanh
            )

            # FC heads: 60 accumulating matmuls over channel c
            # lhsT = emb[:, [c, 60+c]] (actor col, critic col); rhs = fcw[:, c]
            psum_fc = ps.tile([2, FCN], F32, tag="fc")
            emb_r = emb_s[:].rearrange("p (h c) -> p h c", h=2)
            for c in range(C):
                nc.tensor.matmul(
                    psum_fc[:], emb_r[:, :, c], fcw_s[:, c * FCN : (c + 1) * FCN],
                    start=(c == 0), stop=False, skip_group_check=True,
                )
            # extras + bias: lhsT = smalls[:,0:2] [K=4,M=2], rhs = smalls[:,2:103]
            nc.tensor.matmul(
                psum_fc[:], sm_s[:, 0:2], sm_s[:, 2 : 2 + FCN],
                start=False, stop=True, skip_group_check=True,
            )
            fc_s = wk.tile([2, FCN], F32, tag="fcs")
            nc.vector.tensor_copy(fc_s[:], psum_fc[:])
            nc.sync.dma_start(out_ext[:, :], fc_s[:])


def prepare_inputs(
    substrate_features, edge_index, v_cpu_demand_t, v_bw_demand_t,
    num_pending_v_nodes_t, actor_w, actor_b, critic_w, critic_b,
    actor_fc_w, actor_fc_b, critic_fc_w, critic_fc_b,
):
    """Host-side sharding / layout prep. Returns in_maps for the 8 cores."""
    x2 = np.asarray(substrate_features, np.float32)[0]        # [100, F]
    ei = np.asarray(edge_index).astype(np.int64)              # [2, E]
    aw = np.asarray(actor_w, np.float32)                      # [3, F, 60]
    ab = np.asarray(actor_b, np.float32)
    cw = np.asarray(critic_w, np.float32)
    cb = np.asarray(critic_b, np.float32)
    afw = np.asarray(actor_fc_w, np.float32)                  # [6003, 100]
    afb = np.asarray(actor_fc_b, np.float32)
    cfw = np.asarray(critic_fc_w, np.float32)                 # [6003, 1]
    cfb = np.asarray(critic_fc_b, np.float32)
    extras = [
        float(np.asarray(v_cpu_demand_t).reshape(-1)[0]),
        float(np.asarray(v_bw_demand_t).reshape(-1)[0]),
        float(np.asarray(num_pending_v_nodes_t).reshape(-1)[0]),
    ]

    # Dense scaled Laplacian from the edge list (PyG ChebConv, lambda_max=2)
    src, dst = ei[0], ei[1]
    deg = np.bincount(src, minlength=N).astype(np.float32)
    dis = np.where(deg > 0, 1.0 / np.sqrt(np.where(deg > 0, deg, 1.0)), 0.0)
    norm = -(dis[src] * dis[dst]).astype(np.float32)
    L = np.zeros((N, N), np.float32)
    np.add.at(L, (dst, src), norm)
    ltT = np.ascontiguousarray(L.T)                            # lhsT layout

    # Fused conv weights, columns [A0|C0|A1|C1|A2|C2]
    w_all = np.concatenate(
        [aw[0], cw[0], aw[1], cw[1], aw[2], cw[2]], axis=1
    )                                                          # [F, 360]
    xT = np.ascontiguousarray(x2.T)                            # [F, 100]

    # FC weights rearranged: fcw[n, c*101 + a] = actor_fc_w[n*60+c, a],
    # col 100 = critic_fc_w[n*60+c, 0]
    A = afw[:6000].reshape(N, C, ACT)
    Cc = cfw[:6000].reshape(N, C, 1)
    fcw_host = np.ascontiguousarray(
        np.concatenate([A, Cc], axis=2).reshape(N, C * FCN)
    )
    if MM_BF16:
        import ml_dtypes

        fcw_host = fcw_host.astype(ml_dtypes.bfloat16)

    # smalls [4, 224]:
    #  [:, 0:2]      extras lhsT columns (both identical): [v_cpu, v_bw, n_pend, 1]
    #  [:, 2:103]    extras rhs rows: actor_fc_w[6000+j]|critic_fc_w[6000+j];
    #                row 3 = [actor_fc_b | critic_fc_b]
    #  [0, 103:223]  conv bias row [actor_b | critic_b]
    smalls = np.zeros((4, SM_COLS), np.float32)
    for j in range(3):
        smalls[j, 0:2] = extras[j]
        smalls[j, 2 : 2 + ACT] = afw[6000 + j]
        smalls[j, 2 + ACT] = cfw[6000 + j, 0]
    smalls[3, 0:2] = 1.0
    smalls[3, 2 : 2 + ACT] = afb
    smalls[3, 2 + ACT] = cfb[0]
    smalls[0, 103 : 103 + CB] = np.concatenate([ab, cb])

    in_maps = []
    for m in range(N_CORES):
        sl = slice(m * FS, (m + 1) * FS)
        big = np.concatenate([w_all[sl], xT[sl]], axis=1)      # [8192, 460]
        big = np.ascontiguousarray(
            big.reshape(NKT, KT, BB).transpose(1, 0, 2).reshape(128, NKT * BB)
        )
        if MM_BF16:
            import ml_dtypes

            big = big.astype(ml_dtypes.bfloat16)
        in_maps.append(
            {"bigbuf": big, "fcw": fcw_host, "lt": ltT, "smalls": smalls}
        )
    return in_maps


def unshard(results):
    out = np.asarray(results[0]["out"], np.float32)            # [2, 101]
    logits = np.ascontiguousarray(out[0:1, 0:ACT])             # [1, 100]
    values = np.ascontiguousarray(out[1:2, ACT : ACT + 1])     # [1, 1]
    return logits, values


_CACHED = {}


def kernel(**inputs):
    from concourse.bass_utils import run_bass_kernel_spmd

    in_maps = prepare_inputs(**inputs)
    if "nc" not in _CACHED:
        _CACHED["nc"] = build_nc(debug=False)
    res = run_bass_kernel_spmd(
        _CACHED["nc"], in_maps, core_ids=list(range(N_CORES))
    )
    return unshard(res.results)


def run_profiled(in_maps, tmpdir=None, trace=False):
    """Like kernel(), but optionally with NTFF profiling."""
    from concourse.bass_utils import run_bass_kernel_spmd

    if "nc" not in _CACHED:
        _CACHED["nc"] = build_nc(debug=False)
    res = run_bass_kernel_spmd(
        _CACHED["nc"], in_maps, core_ids=list(range(N_CORES)),
        trace=trace, tmpdir=tmpdir,
    )
    return unshard(res.results), res.exec_time_ns, res


# revision 22
# speedup vs baseline: 1.2414x; 1.2414x over previous
"""A3C ChebConv (K=3) GNN model as a distributed Bass kernel on 8 TRN2 cores.

Math restructuring: the reference computes
    Tx0 = x; Tx1 = L@x; Tx2 = 2*L@Tx1 - x
    out = Tx0@W0 + Tx1@W1 + Tx2@W2 + b
Since L acts on the node dim and W on the feature dim, they commute:
    out = Y0 - Y2 + b + L@(Y1 + 2*L@Y2),   Y_k = x@W_k
So the only big compute is x@W (feature contraction, F=65536), which is
sharded over F across 8 cores; the [100, 360] partial products are
all-reduced, and the tiny Laplacian/tanh/FC epilogue runs on every core.

Per-core device graph:
  - one fused matmul  xT_shard[8192,100].T @ W_shard[8192,360] -> PSUM[100,360]
    (64 K-tiles of 128, streamed from one interleaved DRAM buffer)
  - AllReduce[100,360] over 8 cores
  - epilogue: U = L@Y2; Vin = Y1 + 2U; Z = bias + L@Vin + Y0 - Y2;
    emb = tanh(Z); FC heads via 60 accumulating [K=100,M=2]x[K=100,N=101]
    matmuls + one extras/bias matmul -> out[2,101]
"""

import numpy as np

import concourse.bass as bass
import concourse.bacc as bacc
import concourse.mybir as mybir
from concourse import tile
from concourse.tile_rust import add_dep_helper

N_CORES = 8
N = 100          # nodes
F = 65536        # input features
FS = F // N_CORES  # features per core
C = 60           # conv channels per head
CB = 2 * C       # both heads interleaved [actor | critic] per Cheb order
NW = 6 * C       # 360 = fused W columns (3 cheb orders x 2 heads)
BB = NW + N      # 460 = bigbuf row: [W row | xT row]
KT = 128         # contraction tile
NKT = FS // KT   # 64 K tiles per core
CHUNKS = 8       # DMA chunks (NKT/CHUNKS tiles each)
TPC = NKT // CHUNKS
ACT = 100        # action dim
FCN = ACT + 1    # fused FC output cols: [logits | value]
SM_COLS = 224    # smalls tensor cols

F32 = mybir.dt.float32
BF16 = mybir.dt.bfloat16
# Ship the big streamed tensors (x/W interleave + FC weights) in bf16:
# halves HBM traffic and runs the TensorEngine at 1 cycle/row (vs 4 for f32).
MM_BF16 = True
MMDT = BF16 if MM_BF16 else F32
# Wake the collective firmware early with a tiny dummy AllReduce that runs
# under the streaming phase, so the real AllReduce doesn't pay the ~11us
# ncfw wake latency.  (Measured: back-to-back collectives queue badly on
# this stack -- leave off.)
PREWARM_CC = False
# AllReduce the [100,360] partials in bf16 (halves the RDH stage time).
AR_BF16 = True
# K-tiles per DMA chunk, front-loaded small so the TensorEngine starts early.
# Each dma_start costs ~0.7-1.1us of sequencer issue time, so chunk issues
# alternate between the two HWDGE engines (sync=SP, scalar=Activation).
CHUNK_SIZES = [2, 4, 8, 10, 10, 10, 10, 10]
assert sum(CHUNK_SIZES) == NKT


def build_nc(debug: bool = False, reps: int = 1):
    nc = bacc.Bacc(
        "TRN2", target_bir_lowering=False, debug=debug, num_devices=N_CORES
    )
    bigbuf = nc.dram_tensor("bigbuf", [128, NKT * BB], MMDT, kind="ExternalInput")
    fcw = nc.dram_tensor("fcw", [N, C * FCN], MMDT, kind="ExternalInput")
    lt = nc.dram_tensor("lt", [N, N], F32, kind="ExternalInput")
    smalls = nc.dram_tensor("smalls", [4, SM_COLS], F32, kind="ExternalInput")
    out_ext = nc.dram_tensor("out", [2, FCN], F32, kind="ExternalOutput")

    with tile.TileContext(nc) as tc:
        with (
            tc.tile_pool(name="big", bufs=1) as bigpool,
            tc.tile_pool(name="wk", bufs=1) as wk,
            tc.tile_pool(name="ps", bufs=1, space="PSUM") as ps,
            tc.tile_pool(name="dram", bufs=1, space="DRAM") as dram,
        ):
            for _rep in range(reps):
                _build_body(nc, bigpool, wk, ps, dram, bigbuf, fcw, lt, smalls, out_ext)

    nc.compile()
    return nc


def _build_body(nc, bigpool, wk, ps, dram, bigbuf, fcw, lt, smalls, out_ext):
    if True:
        if True:
            # Small persistent tensors, issued on the scalar HWDGE queue and
            # order-pinned into the middle of the DMA ramp: early enough to
            # land before the AllReduce window (they'd contend with the
            # collective's SDMA traffic and stall the FC phase), late enough
            # not to delay the first streaming chunk.
            fcw_s = wk.tile([N, C * FCN], MMDT, tag="fcw")
            i_fcw = nc.scalar.dma_start(fcw_s[:], fcw[:, :])
            lt_s = wk.tile([N, N], F32, tag="lt")
            i_lt = nc.scalar.dma_start(lt_s[:], lt[:, :])
            sm_s = wk.tile([4, SM_COLS], F32, tag="smalls")
            i_sm = nc.scalar.dma_start(sm_s[:], smalls[:, :])
            ones_s = wk.tile([1, N], F32, tag="ones")
            nc.any.memset(ones_s[:], 1.0)

            if PREWARM_CC:
                warm_in = dram.tile([1, 8], F32, tag="warmin")
                warm_out = dram.tile([1, 8], F32, tag="warmout")
                warm_sb = wk.tile([1, 8], F32, tag="warmsb")
                nc.any.memset(warm_sb[:], 0.0)
                nc.gpsimd.dma_start(warm_in[:], warm_sb[:])
                nc.gpsimd.collective_compute(
                    "AllReduce",
                    mybir.AluOpType.add,
                    replica_groups=[list(range(N_CORES))],
                    ins=[warm_in.opt()],
                    outs=[warm_out.opt()],
                )

            # Big fused matmul: accumulate all 64 K-tiles into one PSUM bank
            psum_y = ps.tile([N, NW], F32, tag="y")
            mm = 0
            lo = 0
            chunk_dmas = []
            for ch, tpc in enumerate(CHUNK_SIZES):
                bt = bigpool.tile([128, tpc * BB], MMDT, tag=f"chunk{ch}")
                eng = nc.sync if ch % 2 == 0 else nc.scalar
                i_ch = eng.dma_start(bt[:], bigbuf[:, lo : lo + tpc * BB])
                chunk_dmas.append(i_ch)
                lo += tpc * BB
                for t in range(tpc):
                    base = t * BB
                    lhsT = bt[:, base + NW : base + BB]  # [128, 100] xT tile
                    rhs = bt[:, base : base + NW]        # [128, 360] W tile
                    nc.tensor.matmul(
                        psum_y[:], lhsT, rhs,
                        start=(mm == 0), stop=(mm == NKT - 1),
                    )
                    mm += 1
            # Order the scalar-queue issues: chunk1, chunk3, then the
            # persistent tensors, then the remaining odd chunks.
            scalar_order = [
                chunk_dmas[1], chunk_dmas[3], i_fcw, i_lt, i_sm,
                chunk_dmas[5], chunk_dmas[7],
            ]
            for a, b in zip(scalar_order[1:], scalar_order):
                add_dep_helper(a.ins, b.ins, False, "scalar DMA queue order")

            # Evict partials and AllReduce across the 8 cores
            ardt = BF16 if AR_BF16 else F32
            y_part = wk.tile([N, NW], ardt, tag="ypart")
            nc.vector.tensor_copy(y_part[:], psum_y[:])
            ar_in = dram.tile([N, NW], ardt, tag="arin")
            ar_out = dram.tile([N, NW], ardt, tag="arout")
            nc.sync.dma_start(ar_in[:], y_part[:])
            nc.gpsimd.collective_compute(
                "AllReduce",
                mybir.AluOpType.add,
                replica_groups=[list(range(N_CORES))],
                ins=[ar_in.opt()],
                outs=[ar_out.opt()],
            )
            y_in = wk.tile([N, NW], ardt, tag="yin")
            nc.sync.dma_start(y_in[:], ar_out[:])
            y_s = wk.tile([N, NW], F32, tag="ysb")
            nc.vector.tensor_copy(y_s[:], y_in[:])

            # Epilogue: U = L@Y2
            psum_u = ps.tile([N, CB], F32, tag="u")
            nc.tensor.matmul(
                psum_u[:], lt_s[:], y_s[:, 2 * CB : 3 * CB], start=True, stop=True
            )
            # Vin = 2*U + Y1
            vin_s = wk.tile([N, CB], F32, tag="vin")
            nc.vector.scalar_tensor_tensor(
                vin_s[:], psum_u[:], 2.0, y_s[:, CB : 2 * CB],
                op0=mybir.AluOpType.mult, op1=mybir.AluOpType.add,
            )
            # Z = bias (ones[1,100].T @ biasrow[1,120]) + L@Vin
            psum_z = ps.tile([N, CB], F32, tag="z")
            nc.tensor.matmul(
                psum_z[:], ones_s[:], sm_s[0:1, 103 : 103 + CB],
                start=True, stop=False, skip_group_check=True,
            )
            nc.tensor.matmul(
                psum_z[:], lt_s[:], vin_s[:],
                start=False, stop=True, skip_group_check=True,
            )
            # emb = tanh(Z + Y0 - Y2)
            d_s = wk.tile([N, CB], F32, tag="d")
            nc.vector.tensor_sub(d_s[:], y_s[:, 0:CB], y_s[:, 2 * CB : 3 * CB])
            z_s = wk.tile([N, CB], F32, tag="zs")
            nc.vector.tensor_add(z_s[:], d_s[:], psum_z[:])
            emb_s = wk.tile([N, CB], MMDT, tag="emb")
            nc.scalar.activation(
                emb_s[:], z_s[:], mybir.ActivationFunctionType.Tanh
            )

            # FC heads: 60 accumulating matmuls over channel c
            # lhsT = emb[:, [c, 60+c]] (actor col, critic col); rhs = fcw[:, c]
            psum_fc = ps.tile([2, FCN], F32, tag="fc")
            emb_r = emb_s[:].rearrange("p (h c) -> p h c", h=2)
            for c in range(C):
                nc.tensor.matmul(
                    psum_fc[:], emb_r[:, :, c], fcw_s[:, c * FCN : (c + 1) * FCN],
                    start=(c == 0), stop=False, skip_group_check=True,
                )
            # extras + bias: lhsT = smalls[:,0:2] [K=4,M=2], rhs = smalls[:,2:103]
            nc.tensor.matmul(
                psum_fc[:], sm_s[:, 0:2], sm_s[:, 2 : 2 + FCN],
                start=False, stop=True, skip_group_check=True,
            )
            fc_s = wk.tile([2, FCN], F32, tag="fcs")
            nc.vector.tensor_copy(fc_s[:], psum_fc[:])
            nc.sync.dma_start(out_ext[:, :], fc_s[:])


def prepare_inputs(
    substrate_features, edge_index, v_cpu_demand_t, v_bw_demand_t,
    num_pending_v_nodes_t, actor_w, actor_b, critic_w, critic_b,
    actor_fc_w, actor_fc_b, critic_fc_w, critic_fc_b,
):
    """Host-side sharding / layout prep. Returns in_maps for the 8 cores."""
    x2 = np.asarray(substrate_features, np.float32)[0]        # [100, F]
    ei = np.asarray(edge_index).astype(np.int64)              # [2, E]
    aw = np.asarray(actor_w, np.float32)                      # [3, F, 60]
    ab = np.asarray(actor_b, np.float32)
    cw = np.asarray(critic_w, np.float32)
    cb = np.asarray(critic_b, np.float32)
    afw = np.asarray(actor_fc_w, np.float32)                  # [6003, 100]
    afb = np.asarray(actor_fc_b, np.float32)
    cfw = np.asarray(critic_fc_w, np.float32)                 # [6003, 1]
    cfb = np.asarray(critic_fc_b, np.float32)
    extras = [
        float(np.asarray(v_cpu_demand_t).reshape(-1)[0]),
        float(np.asarray(v_bw_demand_t).reshape(-1)[0]),
        float(np.asarray(num_pending_v_nodes_t).reshape(-1)[0]),
    ]

    # Dense scaled Laplacian from the edge list (PyG ChebConv, lambda_max=2)
    src, dst = ei[0], ei[1]
    deg = np.bincount(src, minlength=N).astype(np.float32)
    dis = np.where(deg > 0, 1.0 / np.sqrt(np.where(deg > 0, deg, 1.0)), 0.0)
    norm = -(dis[src] * dis[dst]).astype(np.float32)
    L = np.zeros((N, N), np.float32)
    np.add.at(L, (dst, src), norm)
    ltT = np.ascontiguousarray(L.T)                            # lhsT layout

    # Fused conv weights, columns [A0|C0|A1|C1|A2|C2]
    w_all = np.concatenate(
        [aw[0], cw[0], aw[1], cw[1], aw[2], cw[2]], axis=1
    )                                                          # [F, 360]
    xT = np.ascontiguousarray(x2.T)                            # [F, 100]

    # FC weights rearranged: fcw[n, c*101 + a] = actor_fc_w[n*60+c, a],
    # col 100 = critic_fc_w[n*60+c, 0]
    A = afw[:6000].reshape(N, C, ACT)
    Cc = cfw[:6000].reshape(N, C, 1)
    fcw_host = np.ascontiguousarray(
        np.concatenate([A, Cc], axis=2).reshape(N, C * FCN)
    )
    if MM_BF16:
        import ml_dtypes

        fcw_host = fcw_host.astype(ml_dtypes.bfloat16)

    # smalls [4, 224]:
    #  [:, 0:2]      extras lhsT columns (both identical): [v_cpu, v_bw, n_pend, 1]
    #  [:, 2:103]    extras rhs rows: actor_fc_w[6000+j]|critic_fc_w[6000+j];
    #                row 3 = [actor_fc_b | critic_fc_b]
    #  [0, 103:223]  conv bias row [actor_b | critic_b]
    smalls = np.zeros((4, SM_COLS), np.float32)
    for j in range(3):
        smalls[j, 0:2] = extras[j]
        smalls[j, 2 : 2 + ACT] = afw[6000 + j]
        smalls[j, 2 + ACT] = cfw[6000 + j, 0]
    smalls[3, 0:2] = 1.0
    smalls[3, 2 : 2 + ACT] = afb
    smalls[3, 2 + ACT] = cfb[0]
    smalls[0, 103 : 103 + CB] = np.concatenate([ab, cb])

    in_maps = []
    for m in range(N_CORES):
        sl = slice(m * FS, (m + 1) * FS)
        big = np.concatenate([w_all[sl], xT[sl]], axis=1)      # [8192, 460]
        big = np.ascontiguousarray(
            big.reshape(NKT, KT, BB).transpose(1, 0, 2).reshape(128, NKT * BB)
        )
        if MM_BF16:
            import ml_dtypes

            big = big.astype(ml_dtypes.bfloat16)
        in_maps.append(
            {"bigbuf": big, "fcw": fcw_host, "lt": ltT, "smalls": smalls}
        )
    return in_maps


def unshard(results):
    out = np.asarray(results[0]["out"], np.float32)            # [2, 101]
    logits = np.ascontiguousarray(out[0:1, 0:ACT])             # [1, 100]
    values = np.ascontiguousarray(out[1:2, ACT : ACT + 1])     # [1, 1]
    return logits, values


_CACHED = {}


def kernel(**inputs):
    from concourse.bass_utils import run_bass_kernel_spmd

    in_maps = prepare_inputs(**inputs)
    if "nc" not in _CACHED:
        _CACHED["nc"] = build_nc(debug=False)
    res = run_bass_kernel_spmd(
        _CACHED["nc"], in_maps, core_ids=list(range(N_CORES))
    )
    return unshard(res.results)


def run_profiled(in_maps, tmpdir=None, trace=False):
    """Like kernel(), but optionally with NTFF profiling."""
    from concourse.bass_utils import run_bass_kernel_spmd

    if "nc" not in _CACHED:
        _CACHED["nc"] = build_nc(debug=False)
    res = run_bass_kernel_spmd(
        _CACHED["nc"], in_maps, core_ids=list(range(N_CORES)),
        trace=trace, tmpdir=tmpdir,
    )
    return unshard(res.results), res.exec_time_ns, res
